# revision 1
# baseline (speedup 1.0000x reference)
"""Trainium2 Bass kernel for the distributed-memory embedding lookup problem.

reference computation (all f32):
    inputs = P[doc_ids] + sum_c W[context_ids]        # [B, D]
    out_vecs = outputs.T[sample_ids]                   # [B, S, D]
    result = einsum("bd,bsd->bs", inputs, out_vecs)    # [B, S]

Strategy: data-parallel over the batch across 8 NeuronCores (2048 rows
each); tables replicated in each core's HBM.  Gathers run as SWDGE
indirect DMAs with one index per partition ([128,1] offset APs — the
HW-validated form; multi-column offset APs gather wrong rows on real
HW).  DVE does the context-sum reduce and the dot products with plain
tensor_tensor / tensor_reduce ops.
"""

import numpy as np

_B = 16384
_C = 8
_S = 10
_D = 128
_N_DOCS = 1_000_000
_N_WORDS = 100_000
_NC = 8
_BPC = _B // _NC        # 2048 batch rows per core
_P = 128                # partitions
_TILES = _BPC // _P     # 16 batch tiles per core
_CHUNK = 4              # batch tiles per pipeline chunk

_nc_cache = {}

LAST_RESULT = None      # BassKernelResults of the most recent run (for test.py)


def _build(n_docs=_N_DOCS, n_words=_N_WORDS, tiles=_TILES, chunk=_CHUNK):
    import concourse.bass as bass
    import concourse.mybir as mybir
    import concourse.tile as tile
    from concourse import bacc

    f32 = mybir.dt.float32
    i32 = mybir.dt.int32

    nc = bacc.Bacc(trn_type="TRN2")
    p_tab = nc.dram_tensor("p_tab", (n_docs, _D), f32, kind="ExternalInput")
    w_tab = nc.dram_tensor("w_tab", (n_words, _D), f32, kind="ExternalInput")
    o_tab = nc.dram_tensor("o_tab", (n_words, _D), f32, kind="ExternalInput")
    doc_idx = nc.dram_tensor("doc_idx", (_P, tiles), i32, kind="ExternalInput")
    ctx_idx = nc.dram_tensor("ctx_idx", (_P, tiles * _C), i32, kind="ExternalInput")
    smp_idx = nc.dram_tensor("smp_idx", (_P, tiles * _S), i32, kind="ExternalInput")
    res = nc.dram_tensor("res", (_P, tiles * _S), f32, kind="ExternalOutput")

    nchunks = tiles // chunk
    with tile.TileContext(nc) as tc:
        with (
            tc.tile_pool(name="idxp", bufs=2) as idxp,
            tc.tile_pool(name="datp", bufs=2) as datp,
            tc.tile_pool(name="resp", bufs=1) as resp,
        ):
            res_sb = resp.tile([_P, tiles * _S], f32)

            # Preload ALL indices once: single-column [128,1] offset slices at
            # any column offset are HW-validated; this removes the per-chunk
            # idx DMAs and their semaphore waits from the Pool stream.
            doc_i = idxp.tile([_P, tiles], i32)
            ctx_i = idxp.tile([_P, tiles * _C], i32)
            smp_i = idxp.tile([_P, tiles * _S], i32)
            nc.sync.dma_start(doc_i[:], doc_idx[:])
            nc.sync.dma_start(ctx_i[:], ctx_idx[:])
            nc.sync.dma_start(smp_i[:], smp_idx[:])

            for ch in range(nchunks):
                p_t = datp.tile([_P, chunk * _D], f32, tag="p")
                w_t = datp.tile([_P, chunk * _C * _D], f32, tag="w")
                o_t = datp.tile([_P, chunk * _S * _D], f32, tag="o")
                # one [128,1]-offset indirect DMA per gathered row-column
                for tt in range(chunk):
                    t = ch * chunk + tt
                    nc.gpsimd.indirect_dma_start(
                        out=p_t[:, tt * _D:(tt + 1) * _D],
                        out_offset=None,
                        in_=p_tab[:],
                        in_offset=bass.IndirectOffsetOnAxis(
                            ap=doc_i[:, t:t + 1], axis=0
                        ),
                    )
                    for c in range(_C):
                        j = tt * _C + c
                        g = t * _C + c
                        nc.gpsimd.indirect_dma_start(
                            out=w_t[:, j * _D:(j + 1) * _D],
                            out_offset=None,
                            in_=w_tab[:],
                            in_offset=bass.IndirectOffsetOnAxis(
                                ap=ctx_i[:, g:g + 1], axis=0
                            ),
                        )
                    for s in range(_S):
                        j = tt * _S + s
                        g = t * _S + s
                        nc.gpsimd.indirect_dma_start(
                            out=o_t[:, j * _D:(j + 1) * _D],
                            out_offset=None,
                            in_=o_tab[:],
                            in_offset=bass.IndirectOffsetOnAxis(
                                ap=smp_i[:, g:g + 1], axis=0
                            ),
                        )

                for tt in range(chunk):
                    t = ch * chunk + tt
                    inp = datp.tile([_P, _D], f32, tag="inp")
                    wv = w_t[:, tt * _C * _D:(tt + 1) * _C * _D].rearrange(
                        "p (c d) -> p d c", c=_C
                    )
                    nc.vector.tensor_reduce(
                        out=inp[:], in_=wv,
                        axis=mybir.AxisListType.X, op=mybir.AluOpType.add,
                    )
                    nc.vector.tensor_add(
                        out=inp[:], in0=inp[:], in1=p_t[:, tt * _D:(tt + 1) * _D]
                    )
                    prod = datp.tile([_P, _S * _D], f32, tag="prod")
                    for s in range(_S):
                        j = tt * _S + s
                        nc.vector.tensor_mul(
                            out=prod[:, s * _D:(s + 1) * _D],
                            in0=o_t[:, j * _D:(j + 1) * _D],
                            in1=inp[:],
                        )
                    pv = prod[:].rearrange("p (s d) -> p s d", s=_S)
                    nc.vector.tensor_reduce(
                        out=res_sb[:, t * _S:(t + 1) * _S], in_=pv,
                        axis=mybir.AxisListType.X, op=mybir.AluOpType.add,
                    )
            nc.sync.dma_start(res[:], res_sb[:])
    nc.compile()
    return nc


def _shard_host(doc_ids, context_ids, sample_ids, tiles=_TILES):
    """Per-core index tensors laid out [128 partitions, ...] so a straight
    DMA lands index (tile t, partition p) at SBUF[p, t]."""
    per_core = []
    bpc = tiles * _P
    ncores = doc_ids.shape[0] // bpc
    for c in range(ncores):
        sl = slice(c * bpc, (c + 1) * bpc)
        d = np.ascontiguousarray(doc_ids[sl].reshape(tiles, _P).T)
        cx = np.ascontiguousarray(
            context_ids[sl].reshape(tiles, _P, _C).transpose(1, 0, 2).reshape(_P, tiles * _C)
        )
        sm = np.ascontiguousarray(
            sample_ids[sl].reshape(tiles, _P, _S).transpose(1, 0, 2).reshape(_P, tiles * _S)
        )
        per_core.append((d, cx, sm))
    return per_core


def kernel(doc_ids, context_ids, sample_ids, paragraph_matrix, word_matrix, outputs):
    global LAST_RESULT
    from concourse.bass_utils import run_bass_kernel_spmd

    doc_ids = np.asarray(doc_ids).astype(np.int32)
    context_ids = np.asarray(context_ids).astype(np.int32)
    sample_ids = np.asarray(sample_ids).astype(np.int32)
    P = np.ascontiguousarray(np.asarray(paragraph_matrix, dtype=np.float32))
    W = np.ascontiguousarray(np.asarray(word_matrix, dtype=np.float32))
    OT = np.ascontiguousarray(np.asarray(outputs, dtype=np.float32).T)  # [N_WORDS, D]

    key = (_N_DOCS, _N_WORDS, _TILES, _CHUNK)
    if key not in _nc_cache:
        _nc_cache[key] = _build()
    nc = _nc_cache[key]

    shards = _shard_host(doc_ids, context_ids, sample_ids)
    in_maps = [
        {"p_tab": P, "w_tab": W, "o_tab": OT, "doc_idx": d, "ctx_idx": cx, "smp_idx": sm}
        for (d, cx, sm) in shards
    ]
    LAST_RESULT = run_bass_kernel_spmd(nc, in_maps, core_ids=list(range(_NC)))

    out = np.empty((_B, _S), dtype=np.float32)
    for c in range(_NC):
        r = LAST_RESULT.results[c]["res"]  # [128, TILES*S]
        out[c * _BPC:(c + 1) * _BPC] = (
            r.reshape(_P, _TILES, _S).transpose(1, 0, 2).reshape(_BPC, _S)
        )
    return out

